# revision 1
# baseline (speedup 1.0000x reference)
"""LSTM autoencoder (B=8192, T=50, F=24; H1=64, LAT=32, H3=64) on 8 trn2 cores.

Data parallel over batch: each core handles Bc=1024 rows. Host transposes x to
[T, F, Bc] (feature-major) so all device DMAs are contiguous, and pre-packs the
LSTM weights as stationary lhsT blobs with the recurrent and input weights
concatenated along the contraction dim (z = [U;W;b]^T @ [h;x;1] in one matmul
per gate pair). Gate math per step (note c >= 0 always since i,f in (0,1) and
g = relu(.) >= 0, so relu(c) == c and the reference's h = o*relu(c) is o*c):

    ps_if = matmul -> sigmoid -> (i | f)         [128, Bc] psum
    ps_go = matmul -> g raw | sigmoid(o)         [128, Bc] psum
    ig = relu(g_raw) * i      (fused scalar_tensor_tensor, one DVE op)
    c  = f*c + ig             (two DVE ops)
    h  = o * c                (one DVE op, writes fp16 straight into the next
                               step's matmul rhs tile)
"""

import os
import sys

import numpy as np

sys.path.insert(0, "/opt/trn_rl_repo")

import concourse.bass as bass
import concourse.mybir as mybir
from concourse.bass_utils import run_bass_kernel_spmd
from concourse.tile import TileContext
from contextlib import ExitStack

B, T, F = 8192, 50, 24
H1, LAT, H3 = 64, 32, 64
NCORES = 8
Bc = B // NCORES  # 1024
HALF = Bc // 2  # max moving free dim per matmul

f16 = mybir.dt.float16
f32 = mybir.dt.float32
AF = mybir.ActivationFunctionType
Alu = mybir.AluOpType

_CACHE = {}

# ---------------------------------------------------------------------------
# Toolchain compat: the walrus build in this container predates two features
# the current Tile framework emits.
#
# 1. Tile's kernel-tail all-engine barrier uses InstEventSemaphore (the EVSEM
#    butterfly), which this walrus cannot codegen (visitInstEventSemaphore
#    throws). Replace it with the legacy 0xD5 PSEUDO_SYNC_BARRIER that NRT
#    expands at load time.
# 2. Tile attaches up to 4 semaphore waits to a single instruction;
#    setupSyncWait here handles exactly one. Split extras into single-wait
#    NoOps prepended on the same engine (engines are in-order, so waiting on
#    the nops first is equivalent).
# ---------------------------------------------------------------------------

bass.Bass.all_engine_barrier = (
    lambda self, *, sem_only=False: self._nrt_pseudo_barrier()
)
bass.Bass.multi_engine_barrier = lambda self, engines: self._nrt_pseudo_barrier()


def _split_multi_waits(js: bytes) -> bytes:
    import json

    m = json.loads(js)
    n_split = 0
    for fn in m["functions"]:
        for blk in fn["blocks"]:
            out = []
            for inst in blk["instructions"]:
                si = inst.get("sync_info")
                waits = (si or {}).get("on_wait") or []
                if len(waits) > 1:
                    for k, w in enumerate(waits[:-1]):
                        out.append(
                            {
                                "name": f"{inst['name']}_w{k}",
                                "engine": inst["engine"],
                                "opcode": "NoOp",
                                "debug": inst.get("debug", 0),
                                "ins": [],
                                "outs": [],
                                "sync_info": {"on_update": [], "on_wait": [w]},
                            }
                        )
                        n_split += 1
                    si["on_wait"] = [waits[-1]]
                out.append(inst)
            blk["instructions"] = out
    return json.dumps(m).encode()


def _wrap_to_json(nc):
    orig = nc.to_json_bytes
    nc.to_json_bytes = lambda: _split_multi_waits(orig())
    return nc


def _build_nc(repeat=1):
    nc = bass.Bass()

    xT_d = nc.dram_tensor("xT", [T, F + 1, Bc], f16, kind="ExternalInput")
    w_if1_d = nc.dram_tensor("w_if1", [H1 + F + 1, 128], f16, kind="ExternalInput")
    w_go1_d = nc.dram_tensor("w_go1", [H1 + F + 1, 128], f16, kind="ExternalInput")
    w_u2_d = nc.dram_tensor("w_u2", [LAT + 1, 128], f16, kind="ExternalInput")
    w_w2_d = nc.dram_tensor("w_w2", [H1, 128], f16, kind="ExternalInput")
    w_if3_d = nc.dram_tensor("w_if3", [H3 + LAT + 1, 128], f16, kind="ExternalInput")
    w_go3_d = nc.dram_tensor("w_go3", [H3 + LAT + 1, 128], f16, kind="ExternalInput")
    w_d_d = nc.dram_tensor("w_d", [H3, F], f16, kind="ExternalInput")
    bd_d = nc.dram_tensor("bd", [F, 1], f32, kind="ExternalInput")
    yT_d = nc.dram_tensor("yT", [T, F, Bc], f32, kind="ExternalOutput")

    K1 = H1 + F + 1  # 89: [h; x; 1]
    K3 = H3 + LAT + 1  # 97: [h; z; 1]

    with TileContext(nc) as tc:
     for _rep in range(repeat):
      with ExitStack() as ctx:
        wp = ctx.enter_context(tc.tile_pool(name=f"wp{_rep}", bufs=1))
        big = ctx.enter_context(tc.tile_pool(name=f"big{_rep}", bufs=1))
        sp = ctx.enter_context(tc.tile_pool(name=f"sp{_rep}", bufs=2))
        pp = ctx.enter_context(tc.tile_pool(name=f"pp{_rep}", bufs=1, space="PSUM"))
        op = ctx.enter_context(tc.tile_pool(name=f"op{_rep}", bufs=3))

        w_if1 = wp.tile([K1, 128], f16)
        nc.sync.dma_start(out=w_if1, in_=w_if1_d[:])
        w_go1 = wp.tile([K1, 128], f16)
        nc.sync.dma_start(out=w_go1, in_=w_go1_d[:])
        w_u2 = wp.tile([LAT + 1, 128], f16)
        nc.sync.dma_start(out=w_u2, in_=w_u2_d[:])
        w_w2 = wp.tile([H1, 128], f16)
        nc.sync.dma_start(out=w_w2, in_=w_w2_d[:])
        w_if3 = wp.tile([K3, 128], f16)
        nc.sync.dma_start(out=w_if3, in_=w_if3_d[:])
        w_go3 = wp.tile([K3, 128], f16)
        nc.sync.dma_start(out=w_go3, in_=w_go3_d[:])
        w_d = wp.tile([H3, F], f16)
        nc.sync.dma_start(out=w_d, in_=w_d_d[:])
        bd = wp.tile([F, 1], f32)
        nc.sync.dma_start(out=bd, in_=bd_d[:])

        halves = (slice(0, HALF), slice(HALF, Bc))

        # ---- LSTM1: cat1[:, t*Bc:(t+1)*Bc] = [h1_{t-1}; x_t^T; 1] ----------
        cat1 = big.tile([K1, (T + 1) * Bc], f16)
        nc.vector.memset(cat1[0:H1, 0:Bc], 0)  # h1_0 = 0
        for t in range(T):
            sl = slice(t * Bc, (t + 1) * Bc)
            nc.sync.dma_start(out=cat1[H1 : H1 + F + 1, sl], in_=xT_d[t])
        c1 = big.tile([H1, Bc], f16)
        nc.vector.memset(c1, 0)

        for t in range(T):
            base = t * Bc
            rhs = cat1[:, base : base + Bc]
            ps_if = pp.tile([128, Bc], f32, tag="ps_if")
            ps_go = pp.tile([128, Bc], f32, tag="ps_go")
            for cs in halves:
                nc.tensor.matmul(ps_if[:, cs], w_if1, rhs[:, cs], start=True, stop=True)
                nc.tensor.matmul(ps_go[:, cs], w_go1, rhs[:, cs], start=True, stop=True)
            # psum layout: ps_if = (f | i), ps_go = (g | o); c1 lives @p0:64.
            sb_if = sp.tile([128, Bc], f16, tag="sb_if")
            sb_o = sp.tile([H1, Bc], f16, tag="sb_o")
            nc.scalar.activation(sb_if, ps_if, AF.Sigmoid)
            nc.scalar.activation(sb_o, ps_go[H1:128, :], AF.Sigmoid)  # cross to @p0
            t_ig = sp.tile([128, Bc], f16, tag="t_ig")
            nc.vector.scalar_tensor_tensor(
                t_ig[H1:128, :], ps_go[0:H1, :], 0.0, sb_if[H1:128, :], Alu.max, Alu.mult
            )
            t_fc = sp.tile([128, Bc], f16, tag="t_fc")
            nc.vector.tensor_mul(t_fc[H1:128, :], sb_if[0:H1, :], c1)
            nc.vector.tensor_add(c1, t_fc[H1:128, :], t_ig[H1:128, :])
            nxt = slice(base + Bc, base + 2 * Bc)
            nc.vector.tensor_mul(cat1[0:H1, nxt], sb_o, c1)

        # ---- LSTM2 (return_sequences=False) --------------------------------
        h2a = big.tile([LAT + 1, Bc], f16)
        h2b = big.tile([LAT + 1, Bc], f16)
        nc.vector.memset(h2a[0:LAT, :], 0)
        nc.vector.memset(h2a[LAT : LAT + 1, :], 1.0)
        nc.vector.memset(h2b[LAT : LAT + 1, :], 1.0)
        c2 = big.tile([LAT, Bc], f16)
        nc.vector.memset(c2, 0)
        h2 = (h2a, h2b)

        for t in range(T):
            cur, nxt = h2[t % 2], h2[(t + 1) % 2]
            h1_t = cat1[0:H1, (t + 1) * Bc : (t + 2) * Bc]
            ps2 = pp.tile([128, Bc], f32, tag="ps2")
            for cs in halves:
                nc.tensor.matmul(ps2[:, cs], w_u2, cur[:, cs], start=True, stop=False)
                nc.tensor.matmul(ps2[:, cs], w_w2, h1_t[:, cs], start=False, stop=True)
            # psum gate order (f | i | o | g); c2 lives @p0:32.
            sb2 = sp.tile([96, Bc], f16, tag="sb2")
            nc.scalar.activation(sb2, ps2[0:96, :], AF.Sigmoid)
            sb_o2 = sp.tile([LAT, Bc], f16, tag="sb_o2")
            nc.vector.tensor_copy(sb_o2, sb2[2 * LAT : 3 * LAT, :])  # o -> @p0
            ig2 = sp.tile([64, Bc], f16, tag="ig2")
            nc.vector.scalar_tensor_tensor(
                ig2[LAT:64, :], ps2[96:128, :], 0.0, sb2[LAT : 2 * LAT, :], Alu.max, Alu.mult
            )
            fc2 = sp.tile([64, Bc], f16, tag="fc2")
            nc.vector.tensor_mul(fc2[LAT:64, :], sb2[0:LAT, :], c2)
            nc.vector.tensor_add(c2, fc2[LAT:64, :], ig2[LAT:64, :])
            nc.vector.tensor_mul(nxt[0:LAT, :], sb_o2, c2)

        z = h2[T % 2][0:LAT, :]

        # ---- LSTM3 + TimeDistributed dense ---------------------------------
        cat3a = big.tile([K3, Bc], f16)
        cat3b = big.tile([K3, Bc], f16)
        nc.vector.memset(cat3a[0:H3, :], 0)  # h3_0 = 0
        nc.vector.tensor_copy(cat3a[H3 : H3 + LAT, :], z)
        nc.vector.tensor_copy(cat3b[H3 : H3 + LAT, :], z)
        nc.vector.memset(cat3a[H3 + LAT : K3, :], 1.0)
        nc.vector.memset(cat3b[H3 + LAT : K3, :], 1.0)
        c3 = big.tile([H3, Bc], f16)
        nc.vector.memset(c3, 0)
        cat3 = (cat3a, cat3b)

        for t in range(T):
            cur, nxt = cat3[t % 2], cat3[(t + 1) % 2]
            ps_if = pp.tile([128, Bc], f32, tag="ps_if")
            ps_go = pp.tile([128, Bc], f32, tag="ps_go")
            for cs in halves:
                nc.tensor.matmul(ps_if[:, cs], w_if3, cur[:, cs], start=True, stop=True)
                nc.tensor.matmul(ps_go[:, cs], w_go3, cur[:, cs], start=True, stop=True)
            sb_if3 = sp.tile([128, Bc], f16, tag="sb_if")
            sb_o3 = sp.tile([H3, Bc], f16, tag="sb_o")
            nc.scalar.activation(sb_if3, ps_if, AF.Sigmoid)
            nc.scalar.activation(sb_o3, ps_go[H3:128, :], AF.Sigmoid)  # cross to @p0
            ig3 = sp.tile([128, Bc], f16, tag="t_ig")
            nc.vector.scalar_tensor_tensor(
                ig3[H3:128, :], ps_go[0:H3, :], 0.0, sb_if3[H3:128, :], Alu.max, Alu.mult
            )
            fc3 = sp.tile([128, Bc], f16, tag="t_fc")
            nc.vector.tensor_mul(fc3[H3:128, :], sb_if3[0:H3, :], c3)
            nc.vector.tensor_add(c3, fc3[H3:128, :], ig3[H3:128, :])
            nc.vector.tensor_mul(nxt[0:H3, :], sb_o3, c3)

            ps_d = pp.tile([F, Bc], f32, tag="ps2")
            for cs in halves:
                nc.tensor.matmul(ps_d[:, cs], w_d, nxt[0:H3, cs], start=True, stop=True)
            yt = op.tile([F, Bc], f32, tag="yt")
            nc.scalar.activation(yt, ps_d, AF.Identity, bias=bd)
            nc.sync.dma_start(out=yT_d[t], in_=yt)

    return nc


def _prep_inputs(inputs):
    """Host-side: shard batch, transpose x, pack weights. Returns in_maps."""
    x = np.asarray(inputs["x"], np.float32)
    W1, U1, b1 = (np.asarray(inputs[k], np.float32) for k in ("W1", "U1", "b1"))
    W2, U2, b2 = (np.asarray(inputs[k], np.float32) for k in ("W2", "U2", "b2"))
    W3, U3, b3 = (np.asarray(inputs[k], np.float32) for k in ("W3", "U3", "b3"))
    Wd, bd = (np.asarray(inputs[k], np.float32) for k in ("Wd", "bd"))

    # lhsT blobs: rows = [U; W; b] so rhs = [h; x; 1]. Gate column order in the
    # reference is (i, f, g, o); (i|f) and (g|o) halves are kept as-is.
    # Device psum gate layouts: L1/L3 half A = (f | i), half B = (g | o);
    # L2 single bank-pair = (f | i | o | g). Reference col order is (i,f,g,o).
    def perm_fi(H):
        return np.concatenate([np.arange(H, 2 * H), np.arange(0, H)])

    uw1 = np.concatenate([U1, W1, b1[None, :]], axis=0)  # [89, 256]
    w_if1 = uw1[:, perm_fi(H1)].astype(np.float16)
    w_go1 = uw1[:, 128:256].astype(np.float16)

    perm2 = np.concatenate(
        [
            np.arange(LAT, 2 * LAT),      # f
            np.arange(0, LAT),            # i
            np.arange(3 * LAT, 4 * LAT),  # o
            np.arange(2 * LAT, 3 * LAT),  # g
        ]
    )
    ub2 = np.concatenate([U2, b2[None, :]], axis=0)  # [33, 128]
    w_u2 = ub2[:, perm2].astype(np.float16)
    w_w2 = W2[:, perm2].astype(np.float16)

    uw3 = np.concatenate([U3, W3, b3[None, :]], axis=0)  # [97, 256]
    w_if3 = uw3[:, perm_fi(H3)].astype(np.float16)
    w_go3 = uw3[:, 128:256].astype(np.float16)

    w_d = Wd.astype(np.float16)
    bd_c = bd.reshape(F, 1).astype(np.float32)

    in_maps = []
    for c in range(NCORES):
        xc = x[c * Bc : (c + 1) * Bc]  # [Bc, T, F]
        xt = xc.transpose(1, 2, 0).astype(np.float16)  # [T, F, Bc]
        xt = np.concatenate([xt, np.ones((T, 1, Bc), np.float16)], axis=1)
        in_maps.append(
            {
                "xT": np.ascontiguousarray(xt),
                "w_if1": w_if1,
                "w_go1": w_go1,
                "w_u2": w_u2,
                "w_w2": w_w2,
                "w_if3": w_if3,
                "w_go3": w_go3,
                "w_d": w_d,
                "bd": bd_c,
            }
        )
    return in_maps


def _make_runner(nc):
    """Compile nc once into a sharded 8-core jit; returns run(in_maps)->results.

    Mirrors bass2jax.run_bass_via_pjrt but caches the compiled executable so
    repeated calls only pay device dispatch.
    """
    import jax
    from jax.sharding import Mesh, PartitionSpec
    from jax.experimental.shard_map import shard_map
    from concourse import bass2jax, mybir as _mb

    bass2jax.install_neuronx_cc_hook()

    partition_name = nc.partition_id_tensor.name if nc.partition_id_tensor else None
    in_names, out_names, out_avals, zero_outs = [], [], [], []
    for alloc in nc.m.functions[0].allocations:
        if not isinstance(alloc, _mb.MemoryLocationSet):
            continue
        name = alloc.memorylocations[0].name
        if alloc.kind == "ExternalInput":
            if name != partition_name:
                in_names.append(name)
        elif alloc.kind == "ExternalOutput":
            out_names.append(name)
            shape = tuple(alloc.tensor_shape)
            dtype = _mb.dt.np(alloc.dtype)
            out_avals.append(jax.core.ShapedArray(shape, dtype))
            zero_outs.append(np.zeros(shape, dtype))
    n_params = len(in_names)
    n_outs = len(out_avals)
    all_in_names = list(in_names) + list(out_names)
    if partition_name is not None:
        all_in_names.append(partition_name)
    donate = tuple(range(n_params, n_params + n_outs))

    def _bind(ins, outs):
        operands = list(ins) + list(outs)
        if partition_name is not None:
            operands.append(bass2jax.partition_id_tensor())
        return bass2jax._bass_exec_p.bind(
            *operands,
            out_avals=tuple(out_avals),
            in_names=tuple(all_in_names),
            out_names=tuple(out_names),
            lowering_input_output_aliases=(),
            sim_require_finite=True,
            sim_require_nnan=True,
            nc=nc,
        )

    def _body(*args):
        return tuple(_bind(args[:n_params], args[n_params:]))

    devices = jax.devices()[:NCORES]
    mesh = Mesh(np.asarray(devices), ("core",))
    in_specs = (PartitionSpec("core"),) * (n_params + n_outs)
    out_specs = (PartitionSpec("core"),) * len(out_names)
    del donate  # kernel writes every output element; skip donation so timing
    # reps can reuse device-resident operands with no H2D re-transfer
    sharded = jax.jit(
        shard_map(
            _body, mesh=mesh, in_specs=in_specs, out_specs=out_specs, check_rep=False
        ),
        keep_unused=True,
    )

    def run(in_maps, timing_reps=0):
        import time as _time
        from jax.sharding import NamedSharding

        sh = NamedSharding(mesh, PartitionSpec("core"))
        concat_in = [
            jax.device_put(
                np.concatenate([np.asarray(m[name]) for m in in_maps], axis=0), sh
            )
            for name in in_names
        ]
        concat_zeros = [
            jax.device_put(np.zeros((NCORES * z.shape[0], *z.shape[1:]), z.dtype), sh)
            for z in zero_outs
        ]
        out_arrs = jax.block_until_ready(sharded(*concat_in, *concat_zeros))
        times = []
        if timing_reps:
            for _ in range(timing_reps):
                t0 = _time.perf_counter()
                jax.block_until_ready(sharded(*concat_in, *concat_zeros))
                times.append(_time.perf_counter() - t0)
        results = [
            {
                name: np.asarray(out_arrs[i]).reshape(NCORES, *out_avals[i].shape)[c]
                for i, name in enumerate(out_names)
            }
            for c in range(NCORES)
        ]
        return results, times

    return run


def _get_runner(repeat=1):
    key = f"runner{repeat}"
    if key not in _CACHE:
        _CACHE[key] = _make_runner(_wrap_to_json(_build_nc(repeat=repeat)))
    return _CACHE[key]


def _run(inputs, trace=False, timing_reps=0):
    in_maps = _prep_inputs(inputs)
    results, times = _get_runner(1)(in_maps, timing_reps=timing_reps)
    y = np.empty((B, T, F), np.float32)
    for c in range(NCORES):
        yt = results[c]["yT"]  # [T, F, Bc]
        y[c * Bc : (c + 1) * Bc] = yt.transpose(2, 0, 1)
    return y, times


def kernel(**inputs):
    y, _ = _run(inputs)
    return y



# revision 4
# speedup vs baseline: 1.2691x; 1.2691x over previous
"""LSTM autoencoder (B=8192, T=50, F=24; H1=64, LAT=32, H3=64) on 8 trn2 cores.

v5: two half-batch streams; LSTM2 lagged two iterations with ping-pong psum
so its extraction chain never threads through LSTM1's; activation passes
whose consumers have slack (relu g1, sigmoid o1, sigmoid f2i2o2) merged to
full FD=1024, while the chain-critical sigmoid(f1,i1) stays per-stream.
fc2 runs on GPSIMD; g2-relu folds into a DVE scalar_tensor_tensor.
"""

import os
import sys

import numpy as np

sys.path.insert(0, "/opt/trn_rl_repo")

import concourse.bass as bass
import concourse.mybir as mybir
from concourse.tile import TileContext
from contextlib import ExitStack

B, T, F = 8192, 50, 24
H1, LAT, H3 = 64, 32, 64
NCORES = 8
Bc = B // NCORES  # 1024
HALF = Bc // 2  # max moving free dim per matmul

f16 = mybir.dt.float16
f32 = mybir.dt.float32
AF = mybir.ActivationFunctionType
Alu = mybir.AluOpType

_CACHE = {}

K1 = H1 + F + 1  # 89:  [h1; x; 1]
KV = 128  # V tile: [h1(0:64); x(64:88); 1(88); pad0(89:96); h2(96:128)]
K3 = 128  # cat3:   [h3(0:64); 1(64); pad0(65:96); z(96:128)]
KD = H3 + 1  # 65:  [h3; 1]
H2OFF = 96  # h2 / z base partition (32-aligned)

# ---------------------------------------------------------------------------
# Toolchain compat: the walrus build in this container predates two features
# the current Tile framework emits (see kernel v1 for details): replace the
# EVSEM tail barrier with the legacy pseudo barrier, and split >1 sem waits
# per instruction into single-wait NoOps.
# ---------------------------------------------------------------------------

bass.Bass.all_engine_barrier = (
    lambda self, *, sem_only=False: self._nrt_pseudo_barrier()
)
bass.Bass.multi_engine_barrier = lambda self, engines: self._nrt_pseudo_barrier()


def _split_multi_waits(js: bytes) -> bytes:
    import json

    m = json.loads(js)
    for fn in m["functions"]:
        for blk in fn["blocks"]:
            out = []
            for inst in blk["instructions"]:
                si = inst.get("sync_info")
                waits = (si or {}).get("on_wait") or []
                if len(waits) > 1:
                    for k, w in enumerate(waits[:-1]):
                        out.append(
                            {
                                "name": f"{inst['name']}_w{k}",
                                "engine": inst["engine"],
                                "opcode": "NoOp",
                                "debug": inst.get("debug", 0),
                                "ins": [],
                                "outs": [],
                                "sync_info": {"on_update": [], "on_wait": [w]},
                            }
                        )
                    si["on_wait"] = [waits[-1]]
                out.append(inst)
            blk["instructions"] = out
    return json.dumps(m).encode()


def _wrap_to_json(nc):
    orig = nc.to_json_bytes
    nc.to_json_bytes = lambda: _split_multi_waits(orig())
    return nc


def _build_nc(repeat=1):
    nc = bass.Bass()

    xT_d = nc.dram_tensor("xT", [T, F + 1, Bc], f16, kind="ExternalInput")
    w_if1_d = nc.dram_tensor("w_if1", [K1, 128], f16, kind="ExternalInput")
    w_og1_d = nc.dram_tensor("w_og1", [K1, 128], f16, kind="ExternalInput")
    w_2_d = nc.dram_tensor("w_2", [KV, 128], f16, kind="ExternalInput")
    w_if3_d = nc.dram_tensor("w_if3", [K3, 128], f16, kind="ExternalInput")
    w_og3_d = nc.dram_tensor("w_og3", [K3, 128], f16, kind="ExternalInput")
    w_d_d = nc.dram_tensor("w_d", [KD, F], f16, kind="ExternalInput")
    yT_d = nc.dram_tensor("yT", [T, F, Bc], f16, kind="ExternalOutput")

    with TileContext(nc) as tc:
     for _rep in range(repeat):
      with ExitStack() as ctx:
        wp = ctx.enter_context(tc.tile_pool(name=f"wp{_rep}", bufs=1))
        st = ctx.enter_context(tc.tile_pool(name=f"st{_rep}", bufs=1))
        sp = ctx.enter_context(tc.tile_pool(name=f"sp{_rep}", bufs=3))
        pp = ctx.enter_context(tc.tile_pool(name=f"pp{_rep}", bufs=1, space="PSUM"))
        op = ctx.enter_context(tc.tile_pool(name=f"op{_rep}", bufs=3))

        w_if1 = wp.tile([K1, 128], f16)
        nc.sync.dma_start(out=w_if1, in_=w_if1_d[:])
        w_og1 = wp.tile([K1, 128], f16)
        nc.sync.dma_start(out=w_og1, in_=w_og1_d[:])
        w_2 = wp.tile([KV, 128], f16)
        nc.sync.dma_start(out=w_2, in_=w_2_d[:])
        w_if3 = wp.tile([K3, 128], f16)
        nc.sync.dma_start(out=w_if3, in_=w_if3_d[:])
        w_og3 = wp.tile([K3, 128], f16)
        nc.sync.dma_start(out=w_og3, in_=w_og3_d[:])
        w_d = wp.tile([KD, F], f16)
        nc.sync.dma_start(out=w_d, in_=w_d_d[:])

        FD = HALF  # 512 per stream
        halves = (slice(0, HALF), slice(HALF, Bc))
        hs = halves

        # ---- shared state (column-sliced per stream) ----------------------
        ring = [
            st.tile([KV, Bc], f16, tag=f"V{i}", name=f"V{i}") for i in range(4)
        ]
        Yg = st.tile([128, Bc], f16, tag="Yg")  # [c1 | g1']
        c2 = st.tile([LAT, Bc], f16, tag="c2")

        nc.vector.memset(ring[0][0:H1, :], 0)  # h1_{-1}
        nc.vector.memset(ring[1][H2OFF:KV, :], 0)  # h2_{-1}
        for rb in ring:  # zero x+pad rows for the 0-weight contraction
            nc.vector.memset(rb[H1:H2OFF, :], 0)
        nc.vector.memset(Yg[0:H1, :], 0)  # c1_0
        nc.vector.memset(c2, 0)  # c2_0

        nc.sync.dma_start(out=ring[0][H1:K1, :], in_=xT_d[0])
        nc.sync.dma_start(out=ring[1][H1:K1, :], in_=xT_d[1])

        # ---- phase A: LSTM1 (t=k) + lagged LSTM2 (t2=k-2) -----------------
        with tc.tile_pool(name=f"ppA{_rep}", bufs=1, space="PSUM") as ppa:
          for k in range(T + 2):
            if k + 2 < T:
                nc.sync.dma_start(
                    out=ring[(k + 2) % 4][H1:K1, :], in_=xT_d[k + 2]
                )

            t2 = k - 2
            if t2 >= 0:
                # LSTM2 step t2: inputs h1_{t2} (pair t2) and h2_{t2-1}
                # (pair k-1) are ready at pair start.
                ps3 = ppa.tile([128, Bc], f32, tag="ps3")
                for cs in hs:
                    nc.tensor.matmul(
                        ps3[:, cs], w_2, ring[(t2 + 1) % 4][0:KV, cs],
                        start=True, stop=True,
                    )
                X2 = sp.tile([96, Bc], f16, tag="X2")
                nc.scalar.activation(X2, ps3[0:96, :], AF.Sigmoid)
                IG2 = sp.tile([LAT, Bc], f16, tag="IG2")
                nc.vector.scalar_tensor_tensor(
                    IG2, ps3[96:128, :], 0.0, X2[LAT : 2 * LAT, :],
                    Alu.max, Alu.mult,
                )  # ig2 = relu(g2) * si2  (psum+sbuf: bases may differ)
                FC2 = sp.tile([LAT, Bc], f16, tag="FC2")
                OC2 = sp.tile([LAT, Bc], f16, tag="OC2")
                for cs in hs:
                    nc.vector.tensor_copy(OC2[:, cs], X2[2 * LAT : 96, cs])
                    nc.gpsimd.tensor_tensor(
                        out=FC2[:, cs], in0=X2[0:LAT, cs], in1=c2[:, cs],
                        op=Alu.mult,
                    )  # fc2
                    nc.gpsimd.tensor_tensor(
                        out=c2[:, cs], in0=FC2[:, cs], in1=IG2[:, cs],
                        op=Alu.add,
                    )  # c2'
                    nc.vector.tensor_mul(
                        ring[(t2 + 2) % 4][H2OFF:KV, cs],
                        OC2[:, cs], c2[:, cs],
                    )  # h2_t2

            if k < T:
                V = ring[k % 4]
                Vn = ring[(k + 1) % 4]
                ps1 = ppa.tile([128, Bc], f32, tag="ps1")
                ps2 = ppa.tile([128, Bc], f32, tag="ps2")
                for cs in hs:
                    nc.tensor.matmul(
                        ps1[:, cs], w_if1, V[0:K1, cs], start=True, stop=True
                    )
                    nc.tensor.matmul(
                        ps2[:, cs], w_og1, V[0:K1, cs], start=True, stop=True
                    )
                nc.scalar.activation(Yg[H1:128, :], ps2[H1:128, :], AF.Relu)
                S1s = []
                for s, cs in enumerate(hs):
                    S1 = sp.tile(
                        [128, FD], f16, tag=f"S1{s}", name=f"S1{s}"
                    )
                    nc.scalar.activation(S1, ps1[:, cs], AF.Sigmoid)
                    S1s.append(S1)
                X1 = sp.tile([H1, Bc], f16, tag="X1")
                nc.scalar.activation(X1, ps2[0:H1, :], AF.Sigmoid)  # so1

                FI = sp.tile([128, Bc], f16, tag="FI")
                FIc = sp.tile([H1, Bc], f16, tag="FIc")
                for s, cs in enumerate(hs):
                    nc.vector.tensor_mul(FI[:, cs], Yg[:, cs], S1s[s])
                    nc.vector.tensor_copy(FIc[:, cs], FI[H1:128, cs])
                    nc.vector.tensor_add(
                        Yg[0:H1, cs], FI[0:H1, cs], FIc[:, cs]
                    )  # c1'
                    nc.vector.tensor_mul(
                        Vn[0:H1, cs], X1[:, cs], Yg[0:H1, cs]
                    )  # h1_k

        # ---- z = h2_{T-1} -> cat3 tiles -----------------------------------
        CAs, CBs, Yg3s = [], [], []
        for s in range(2):
            z_src = ring[(T + 1) % 4][H2OFF:KV, hs[s]]
            CA = st.tile([K3, FD], f16, tag=f"CA{s}", name=f"CA{s}")
            CB = st.tile([K3, FD], f16, tag=f"CB{s}", name=f"CB{s}")
            nc.vector.memset(CA[0:H3, :], 0)  # h3_{-1}
            nc.vector.memset(CA[H3:H2OFF, :], 0)
            nc.vector.memset(CB[H3:H2OFF, :], 0)
            nc.vector.memset(CA[H3 : H3 + 1, :], 1.0)
            nc.vector.memset(CB[H3 : H3 + 1, :], 1.0)
            nc.vector.tensor_copy(CA[H2OFF:K3, :], z_src)
            nc.vector.tensor_copy(CB[H2OFF:K3, :], z_src)
            Yg3 = st.tile([128, FD], f16, tag=f"Yg3{s}", name=f"Yg3{s}")
            nc.vector.memset(Yg3[0:H3, :], 0)
            CAs.append(CA)
            CBs.append(CB)
            Yg3s.append(Yg3)

        # ---- phase B: LSTM3 + dense, two interleaved streams --------------
        with tc.tile_pool(name=f"ppB{_rep}", bufs=1, space="PSUM") as ppb:
          for t in range(T):
            for s in range(2):
                cat3 = (CAs[s], CBs[s])
                Yg3 = Yg3s[s]
                C = cat3[t % 2]
                Cn = cat3[(t + 1) % 2]
                ps4 = ppb.tile([128, FD], f32, tag=f"ps4{s}", name=f"ps4{s}")
                ps5 = ppb.tile([128, FD], f32, tag=f"ps5{s}", name=f"ps5{s}")
                nc.tensor.matmul(ps4, w_if3, C[0:K3, :], start=True, stop=True)
                nc.tensor.matmul(ps5, w_og3, C[0:K3, :], start=True, stop=True)

                S3 = sp.tile([128, FD], f16, tag=f"S1{s}", name=f"S3{s}")
                nc.scalar.activation(S3, ps4, AF.Sigmoid)  # (sf3 | si3)
                nc.scalar.activation(Yg3[H3:128, :], ps5[H3:128, :], AF.Relu)
                X3 = sp.tile([H3, FD], f16, tag=f"X1{s}", name=f"X3{s}")
                nc.scalar.activation(X3, ps5[0:H3, :], AF.Sigmoid)  # so3

                FI3 = sp.tile([128, FD], f16, tag=f"FI{s}", name=f"FI3{s}")
                FIc3 = sp.tile([H3, FD], f16, tag=f"FIc{s}", name=f"FIc3{s}")
                nc.vector.tensor_mul(FI3, Yg3, S3)
                nc.vector.tensor_copy(FIc3, FI3[H3:128, :])
                nc.vector.tensor_add(Yg3[0:H3, :], FI3[0:H3, :], FIc3)
                nc.vector.tensor_mul(Cn[0:H3, :], X3, Yg3[0:H3, :])  # h3_t

                ps_d = ppb.tile([F, FD], f32, tag=f"psd{s}", name=f"psd{s}")
                nc.tensor.matmul(ps_d, w_d, Cn[0:KD, :], start=True, stop=True)
                yt = op.tile([F, FD], f16, tag=f"yt{s}", name=f"yt{s}")
                nc.vector.tensor_copy(yt, ps_d)
                nc.sync.dma_start(out=yT_d[t][:, hs[s]], in_=yt)

    return nc


def _prep_inputs(inputs):
    """Host-side: shard batch, transpose x, pack weights. Returns in_maps."""
    x = np.asarray(inputs["x"], np.float32)
    W1, U1, b1 = (np.asarray(inputs[k], np.float32) for k in ("W1", "U1", "b1"))
    W2, U2, b2 = (np.asarray(inputs[k], np.float32) for k in ("W2", "U2", "b2"))
    W3, U3, b3 = (np.asarray(inputs[k], np.float32) for k in ("W3", "U3", "b3"))
    Wd, bd = (np.asarray(inputs[k], np.float32) for k in ("Wd", "bd"))

    # Reference gate column order is (i, f, g, o), each H wide.
    def cols(H, *gates):
        idx = {"i": 0, "f": 1, "g": 2, "o": 3}
        return np.concatenate([np.arange(idx[g] * H, (idx[g] + 1) * H) for g in gates])

    uw1 = np.concatenate([U1, W1, b1[None, :]], axis=0)  # [89, 256]
    w_if1 = uw1[:, cols(H1, "f", "i")].astype(np.float16)
    w_og1 = uw1[:, cols(H1, "o", "g")].astype(np.float16)

    # V rows: [h1(64); x(24); 1; pad(7); h2(32)] -> [W2; 0; b2; 0; U2]
    w_2 = np.concatenate(
        [
            W2,
            np.zeros((F, 4 * LAT), np.float32),
            b2[None, :],
            np.zeros((7, 4 * LAT), np.float32),
            U2,
        ],
        axis=0,
    )[:, cols(LAT, "f", "i", "o", "g")].astype(np.float16)

    # cat3 rows: [h3(64); 1; pad(31); z(32)] -> [U3; b3; 0; W3]
    uw3 = np.concatenate(
        [U3, b3[None, :], np.zeros((31, 4 * H3), np.float32), W3], axis=0
    )  # [128, 256]
    w_if3 = uw3[:, cols(H3, "f", "i")].astype(np.float16)
    w_og3 = uw3[:, cols(H3, "o", "g")].astype(np.float16)

    w_d = np.concatenate([Wd, bd[None, :]], axis=0).astype(np.float16)  # [65, 24]

    in_maps = []
    for c in range(NCORES):
        xc = x[c * Bc : (c + 1) * Bc]  # [Bc, T, F]
        xt = xc.transpose(1, 2, 0).astype(np.float16)  # [T, F, Bc]
        xt = np.concatenate([xt, np.ones((T, 1, Bc), np.float16)], axis=1)
        in_maps.append(
            {
                "xT": np.ascontiguousarray(xt),
                "w_if1": w_if1,
                "w_og1": w_og1,
                "w_2": w_2,
                "w_if3": w_if3,
                "w_og3": w_og3,
                "w_d": w_d,
            }
        )
    return in_maps


def _make_runner(nc):
    """Compile nc once into a sharded 8-core jit; returns run(in_maps)->results."""
    import jax
    from jax.sharding import Mesh, PartitionSpec
    from jax.experimental.shard_map import shard_map
    from concourse import bass2jax, mybir as _mb

    bass2jax.install_neuronx_cc_hook()

    partition_name = nc.partition_id_tensor.name if nc.partition_id_tensor else None
    in_names, out_names, out_avals, zero_outs = [], [], [], []
    for alloc in nc.m.functions[0].allocations:
        if not isinstance(alloc, _mb.MemoryLocationSet):
            continue
        name = alloc.memorylocations[0].name
        if alloc.kind == "ExternalInput":
            if name != partition_name:
                in_names.append(name)
        elif alloc.kind == "ExternalOutput":
            out_names.append(name)
            shape = tuple(alloc.tensor_shape)
            dtype = _mb.dt.np(alloc.dtype)
            out_avals.append(jax.core.ShapedArray(shape, dtype))
            zero_outs.append(np.zeros(shape, dtype))
    n_params = len(in_names)
    n_outs = len(out_avals)
    all_in_names = list(in_names) + list(out_names)
    if partition_name is not None:
        all_in_names.append(partition_name)

    def _bind(ins, outs):
        operands = list(ins) + list(outs)
        if partition_name is not None:
            operands.append(bass2jax.partition_id_tensor())
        return bass2jax._bass_exec_p.bind(
            *operands,
            out_avals=tuple(out_avals),
            in_names=tuple(all_in_names),
            out_names=tuple(out_names),
            lowering_input_output_aliases=(),
            sim_require_finite=True,
            sim_require_nnan=True,
            nc=nc,
        )

    def _body(*args):
        return tuple(_bind(args[:n_params], args[n_params:]))

    devices = jax.devices()[:NCORES]
    mesh = Mesh(np.asarray(devices), ("core",))
    in_specs = (PartitionSpec("core"),) * (n_params + n_outs)
    out_specs = (PartitionSpec("core"),) * len(out_names)
    sharded = jax.jit(
        shard_map(
            _body, mesh=mesh, in_specs=in_specs, out_specs=out_specs, check_rep=False
        ),
        keep_unused=True,
    )

    def prepare(in_maps):
        from jax.sharding import NamedSharding

        sh = NamedSharding(mesh, PartitionSpec("core"))
        concat_in = [
            jax.device_put(
                np.concatenate([np.asarray(m[name]) for m in in_maps], axis=0), sh
            )
            for name in in_names
        ]
        concat_zeros = [
            jax.device_put(np.zeros((NCORES * z.shape[0], *z.shape[1:]), z.dtype), sh)
            for z in zero_outs
        ]
        return concat_in, concat_zeros

    def execute(args):
        concat_in, concat_zeros = args
        return jax.block_until_ready(sharded(*concat_in, *concat_zeros))

    def run(in_maps, timing_reps=0):
        import time as _time

        concat_in, concat_zeros = prepare(in_maps)
        out_arrs = jax.block_until_ready(sharded(*concat_in, *concat_zeros))
        times = []
        if timing_reps:
            for _ in range(timing_reps):
                t0 = _time.perf_counter()
                jax.block_until_ready(sharded(*concat_in, *concat_zeros))
                times.append(_time.perf_counter() - t0)
        results = [
            {
                name: np.asarray(out_arrs[i]).reshape(NCORES, *out_avals[i].shape)[c]
                for i, name in enumerate(out_names)
            }
            for c in range(NCORES)
        ]
        return results, times

    run.prepare = prepare
    run.execute = execute
    return run


def _get_runner(repeat=1):
    key = f"runner{repeat}"
    if key not in _CACHE:
        _CACHE[key] = _make_runner(_wrap_to_json(_build_nc(repeat=repeat)))
    return _CACHE[key]


def _run(inputs, trace=False, timing_reps=0):
    in_maps = _prep_inputs(inputs)
    results, times = _get_runner(1)(in_maps, timing_reps=timing_reps)
    y = np.empty((B, T, F), np.float32)
    for c in range(NCORES):
        yt = results[c]["yT"].astype(np.float32)  # [T, F, Bc]
        y[c * Bc : (c + 1) * Bc] = yt.transpose(2, 0, 1)
    return y, times


def kernel(**inputs):
    y, _ = _run(inputs)
    return y
